# revision 16
# baseline (speedup 1.0000x reference)
"""Trainium2 Bass kernel for a dense transformer block (B=2, T=2048, C=1024, nh=16, H=4096).

Strategy (8 NeuronCores, no device collectives -- they measure ~300us for an 8MB
quad AllReduce here, far more than the whole compute budget):

  Launch 1 (head-parallel): cores 0-3 <- batch 0, cores 4-7 <- batch 1; each core
    handles 4 attention heads over the full sequence. x arrives pre-transposed
    (feature-major); LN1 statistics are computed with ones-vector matmuls on the
    TensorE and applied with broadcast vector ops, so no on-device transposes are
    needed. QKV run as fp32r matmuls (feature-major Q/K, token-major V with a
    ones column appended so the softmax denominator falls out of the AV matmul).
    Causal attention uses 512-token query chunks with the key-tile loop outer;
    exp on ScalarE, diagonal-block multiplicative masks, denominator divide on
    VectorE after a GpSimd partition broadcast. Output: row-parallel c_proj
    partial [2048, 1024].

  Host: pure re-slicing of the partials (no arithmetic).

  Launch 2 (token-parallel): each core takes a 512-token slice: sums the 4 proj
    partials on-device, + residual + proj_b -> LN2 -> c_fc (feature-major hidden)
    -> gaussian activation (2 ScalarE passes; mu/sigma/fc_b folded into the
    activation bias/scale, gamma/beta folded into fc2 weights/bias on host)
    -> c_fc2 -> + residual -> final output slice. MLP weights stream from HBM in
    contiguous per-chunk layouts to keep the DMA engines at line rate.

Both launches are uniform SPMD programs (same instruction stream on all 8 cores,
different data), run via run_bass_kernel_spmd.
"""

import hashlib
import os
import shutil
from contextlib import ExitStack

import numpy as np

import concourse.bass as bass
import concourse.tile as tile
from concourse import bacc, mybir
from concourse.bass_utils import run_bass_kernel_spmd

F32 = mybir.dt.float32
F32R = mybir.dt.float32r
AF = mybir.ActivationFunctionType
ALU = mybir.AluOpType

N_CORES = 8
T = 2048          # tokens per batch
C = 1024          # model dim
NH_LOC = 4        # heads per core (launch 1)
HS = 64           # head size
HID = 4096        # mlp hidden
TS = 512          # tokens per core (launch 2)

LAST_EXEC_NS = {}  # launch name -> exec_time_ns (filled when tracing enabled)

_CACHE_DIR = "/tmp/neff_cache"


def _install_compile_cache():
    import concourse.bass2jax as b2j

    if getattr(b2j, "_neff_cache_installed", False):
        return
    real = b2j.compile_bir_kernel

    def cached(bir_json, tmpdir, neff_name="file.neff"):
        os.makedirs(_CACHE_DIR, exist_ok=True)
        h = hashlib.sha256(bir_json).hexdigest()
        cpath = os.path.join(_CACHE_DIR, h + ".neff")
        out = os.path.join(tmpdir, neff_name)
        if os.path.exists(cpath):
            shutil.copyfile(cpath, out)
            return out
        res = real(bir_json, tmpdir, neff_name)
        shutil.copyfile(res, cpath)
        return res

    b2j.compile_bir_kernel = cached
    b2j._neff_cache_installed = True


# --------------------------------------------------------------------------
# Launch 1: LN1 + QKV + causal attention (4 heads) + c_proj partial
# --------------------------------------------------------------------------
def build_l1(ln1_apply: bool):
    nc = bacc.Bacc("TRN2", target_bir_lowering=False, debug=False,
                   num_devices=N_CORES)
    xT_d = nc.dram_tensor("xT", [128, 8, T], F32R, kind="ExternalInput")
    wqk_d = nc.dram_tensor("wqkT", [128, 8, 512], F32R, kind="ExternalInput")
    wv_d = nc.dram_tensor("wvT", [128, 8, 256], F32R, kind="ExternalInput")
    bqk_d = nc.dram_tensor("bqk", [128, 4], F32, kind="ExternalInput")
    bv_d = nc.dram_tensor("bv", [1, 256], F32, kind="ExternalInput")
    pw_d = nc.dram_tensor("projwT", [128, 2, 1024], F32R, kind="ExternalInput")
    mask_d = nc.dram_tensor("masks", [128, 4, 512], F32R, kind="ExternalInput")
    vones_d = nc.dram_tensor("vones", [128, 64], F32R, kind="ExternalInput")
    if ln1_apply:
        lnw_d = nc.dram_tensor("ln1w", [128, 8], F32, kind="ExternalInput")
        lnb_d = nc.dram_tensor("ln1b", [128, 8], F32, kind="ExternalInput")
    yp_d = nc.dram_tensor("yp", [T, C], F32, kind="ExternalOutput")

    n_ttiles = T // 128          # 16
    n_tc = T // 512              # 4 attention query chunks

    with tile.TileContext(nc) as tc, ExitStack() as ctx:
        consts = ctx.enter_context(tc.tile_pool(name="consts", bufs=1))
        vones_sb = consts.tile([128, 64], F32R)
        nc.sync.dma_start(out=vones_sb[:], in_=vones_d[:])
        eps_sb = consts.tile([128, 1], F32)
        nc.vector.memset(eps_sb[:], 1e-5)
        bqk_sb = consts.tile([128, 4], F32)
        nc.sync.dma_start(out=bqk_sb[:], in_=bqk_d[:])
        bv_row = consts.tile([1, 256], F32)
        nc.sync.dma_start(out=bv_row[:], in_=bv_d[:])
        bv_b = consts.tile([128, 256], F32)
        nc.gpsimd.partition_broadcast(bv_b[:], bv_row[:])
        if ln1_apply:
            lnw_sb = consts.tile([128, 8], F32)
            nc.sync.dma_start(out=lnw_sb[:], in_=lnw_d[:])
            lnb_sb = consts.tile([128, 8], F32)
            nc.sync.dma_start(out=lnb_sb[:], in_=lnb_d[:])

        big = ctx.enter_context(tc.tile_pool(name="big", bufs=1))
        qkT = big.tile([128, 4, T], F32R)      # Q feats (tiles 0,1), K feats (2,3)
        v_sb = big.tile([128, n_ttiles, NH_LOC, 65], F32R)
        yT = big.tile([128, 2, T], F32R)
        pw_sb = big.tile([128, 2, 1024], F32R)

        wpool = ctx.enter_context(tc.tile_pool(name="wpool", bufs=1))
        wqk_sb = wpool.tile([128, 8, 512], F32R)
        nc.sync.dma_start(out=wqk_sb[:], in_=wqk_d[:])
        wv_sb = wpool.tile([128, 8, 256], F32R)
        nc.sync.dma_start(out=wv_sb[:], in_=wv_d[:])
        bigh_cm = tc.tile_pool(name="bigh", bufs=1)
        bigh = bigh_cm.__enter__()
        h1T = bigh.tile([128, 8, T], F32R)     # 64KB/p, freed after P2

        # ---- P1: LN1 in transposed space (x streamed twice) ----
        # sums/sumsq via ones-vector matmuls; normalize with broadcast DVE ops.
        with tc.tile_pool(name="p1", bufs=3) as p1, \
             tc.tile_pool(name="p1b", bufs=2) as p1b, \
             tc.tile_pool(name="p1r", bufs=6) as p1r, \
             tc.tile_pool(name="p1psum", bufs=2, space="PSUM") as p1p:
            for tch in range(n_tc):
                sl = slice(tch * 512, (tch + 1) * 512)
                ps_sum = p1p.tile([1, 512], F32, tag="s")
                ps_sq = p1p.tile([1, 512], F32, tag="q")
                for c in range(8):
                    xt = p1.tile([128, 512], F32R, tag="xt")
                    nc.sync.dma_start(out=xt[:], in_=xT_d[:, c, sl])
                    sq = p1.tile([128, 512], F32R, tag="sq")
                    nc.vector.tensor_mul(sq[:], xt[:], xt[:])
                    nc.tensor.matmul(ps_sum[:], vones_sb[:, 0:1], xt[:],
                                     start=(c == 0), stop=(c == 7),
                                     skip_group_check=True)
                    nc.tensor.matmul(ps_sq[:], vones_sb[:, 0:1], sq[:],
                                     start=(c == 0), stop=(c == 7),
                                     skip_group_check=True)
                srow = p1r.tile([1, 512], F32, tag="row")
                nc.vector.tensor_copy(srow[:], ps_sum[:])
                qrow = p1r.tile([1, 512], F32, tag="row")
                nc.vector.tensor_copy(qrow[:], ps_sq[:])
                mrow = p1r.tile([1, 512], F32, tag="row")
                nc.vector.tensor_scalar(out=mrow[:], in0=srow[:], scalar1=1.0 / C,
                                        scalar2=None, op0=ALU.mult)
                msq = p1r.tile([1, 512], F32, tag="row")
                nc.vector.tensor_mul(msq[:], mrow[:], mrow[:])
                vrow = p1r.tile([1, 512], F32, tag="row")
                nc.vector.scalar_tensor_tensor(out=vrow[:], in0=qrow[:],
                                               scalar=1.0 / C, in1=msq[:],
                                               op0=ALU.mult, op1=ALU.subtract)
                sdr = p1r.tile([1, 512], F32, tag="row")
                nc.scalar.activation(out=sdr[:], in_=vrow[:], func=AF.Sqrt,
                                     bias=eps_sb[0:1], scale=1.0)
                rrow = p1r.tile([1, 512], F32, tag="row")
                nc.vector.reciprocal(rrow[:], sdr[:])
                mb = p1b.tile([128, 512], F32, tag="mb")
                nc.gpsimd.partition_broadcast(mb[:], mrow[:])
                rb = p1b.tile([128, 512], F32, tag="rb")
                nc.gpsimd.partition_broadcast(rb[:], rrow[:])
                for c in range(8):
                    xt2 = p1.tile([128, 512], F32R, tag="xt")
                    nc.sync.dma_start(out=xt2[:], in_=xT_d[:, c, sl])
                    t1 = p1.tile([128, 512], F32, tag="t1")
                    nc.vector.tensor_sub(t1[:], xt2[:], mb[:])
                    if ln1_apply:
                        t2 = p1.tile([128, 512], F32, tag="t2")
                        nc.vector.tensor_mul(t2[:], t1[:], rb[:])
                        nc.vector.tensor_scalar(out=h1T[:, c, sl], in0=t2[:],
                                                scalar1=lnw_sb[:, c:c + 1],
                                                scalar2=lnb_sb[:, c:c + 1],
                                                op0=ALU.mult, op1=ALU.add)
                    else:
                        nc.vector.tensor_mul(h1T[:, c, sl], t1[:], rb[:])

        # ---- P2: QKV projections ----
        with tc.tile_pool(name="p2psum", bufs=3, space="PSUM") as p2p:
            for f in range(4):
                for tch in range(n_tc):
                    ps = p2p.tile([128, 512], F32, tag="qk")
                    for c in range(8):
                        nc.tensor.matmul(
                            ps[:], wqk_sb[:, c, f * 128:(f + 1) * 128],
                            h1T[:, c, tch * 512:(tch + 1) * 512],
                            start=(c == 0), stop=(c == 7))
                    sc = 0.125 if f < 2 else 1.0
                    nc.vector.tensor_scalar(
                        out=qkT[:, f, tch * 512:(tch + 1) * 512], in0=ps[:],
                        scalar1=bqk_sb[:, f:f + 1], scalar2=sc,
                        op0=ALU.add, op1=ALU.mult)
            for tt in range(n_ttiles):
                ps = p2p.tile([128, 256], F32, tag="v")
                for c in range(8):
                    nc.tensor.matmul(
                        ps[:], h1T[:, c, tt * 128:(tt + 1) * 128],
                        wv_sb[:, c, :], start=(c == 0), stop=(c == 7))
                for h in range(NH_LOC):
                    nc.vector.tensor_add(v_sb[:, tt, h, 0:64],
                                         ps[:, h * 64:(h + 1) * 64],
                                         bv_b[:, h * 64:(h + 1) * 64])
            nc.sync.dma_start(out=v_sb[:, :, :, 64:65], in_=vones_d[:])

        bigh_cm.__exit__(None, None, None)
        nc.sync.dma_start(out=pw_sb[:], in_=pw_d[:])

        # ---- P3: attention (key-tile outer, 512-token query chunks) ----
        with tc.tile_pool(name="p3consts", bufs=1) as p3c, \
             tc.tile_pool(name="p3a", bufs=4) as p3a, \
             tc.tile_pool(name="p3s", bufs=3) as p3s, \
             tc.tile_pool(name="p3ps", bufs=3, space="PSUM") as p3ps, \
             tc.tile_pool(name="p3py", bufs=1, space="PSUM") as p3py:
            mask_sb = p3c.tile([128, 4, 512], F32R)
            nc.sync.dma_start(out=mask_sb[:], in_=mask_d[:])
            for h in range(NH_LOC):
                po = (h % 2) * 64
                qf = h // 2
                kf = 2 + h // 2
                pys = []
                for tcx in range(n_tc):
                    py_t = p3py.tile([65, 512], F32, tag=f"py{tcx}",
                                     name=f"py{h}_{tcx}")
                    pys.append(py_t)
                for s in range(n_ttiles):
                    for tcx in range(s // 4, n_tc):
                        qsl = slice(tcx * 512, (tcx + 1) * 512)
                        pscore = p3ps.tile([128, 512], F32, tag="sc")
                        nc.tensor.matmul(
                            pscore[:],
                            qkT[po:po + 64, kf, s * 128:(s + 1) * 128],
                            qkT[po:po + 64, qf, qsl],
                            start=True, stop=True)
                        at = p3a.tile([128, 512], F32R, tag="at")
                        nc.scalar.activation(out=at[:], in_=pscore[:], func=AF.Exp)
                        if tcx == s // 4:
                            nc.vector.tensor_mul(at[:], at[:],
                                                 mask_sb[:, s % 4, :])
                        nc.tensor.matmul(pys[tcx][:], v_sb[:, s, h, :], at[:],
                                         start=(s == 0), stop=(s == 4 * tcx + 3),
                                         skip_group_check=True)
                        if s == 4 * tcx + 3:
                            yc = p3s.tile([65, 512], F32, tag="yc")
                            nc.vector.tensor_copy(yc[:], pys[tcx][:])
                            rn = p3s.tile([1, 512], F32, tag="rn")
                            nc.vector.reciprocal(rn[:], yc[64:65, :])
                            db = p3s.tile([64, 512], F32, tag="db")
                            nc.gpsimd.partition_broadcast(db[:], rn[:])
                            nc.vector.tensor_mul(
                                yT[po:po + 64, h // 2, qsl],
                                yc[0:64, :], db[:])

        # ---- P4: c_proj partial ----
        with tc.tile_pool(name="p4o", bufs=3) as p4o, \
             tc.tile_pool(name="p4ps", bufs=2, space="PSUM") as p4p:
            for tt in range(n_ttiles):
                for co in range(2):
                    pp = p4p.tile([128, 512], F32)
                    for cl in range(2):
                        nc.tensor.matmul(
                            pp[:], yT[:, cl, tt * 128:(tt + 1) * 128],
                            pw_sb[:, cl, co * 512:(co + 1) * 512],
                            start=(cl == 0), stop=(cl == 1))
                    ot = p4o.tile([128, 512], F32)
                    nc.vector.tensor_copy(ot[:], pp[:])
                    nc.sync.dma_start(
                        out=yp_d[tt * 128:(tt + 1) * 128,
                                 co * 512:(co + 1) * 512], in_=ot[:])
    nc.compile()
    return nc


# --------------------------------------------------------------------------
# Launch 2: reduce partials + residual + LN2 + MLP + residual
# --------------------------------------------------------------------------
def build_l2(ln2_apply: bool, s_act: float):
    nc = bacc.Bacc("TRN2", target_bir_lowering=False, debug=False,
                   num_devices=N_CORES)
    yp4_d = nc.dram_tensor("yp4", [4, TS, C], F32, kind="ExternalInput")
    xs_d = nc.dram_tensor("xs", [TS, C], F32, kind="ExternalInput")
    pb_d = nc.dram_tensor("pb", [1, C], F32, kind="ExternalInput")
    fb2_d = nc.dram_tensor("fb2", [1, C], F32, kind="ExternalInput")
    ab_d = nc.dram_tensor("abias", [128, 32], F32, kind="ExternalInput")
    fcw_d = nc.dram_tensor("fcwT", [8, 128, 8, 512], F32R, kind="ExternalInput")
    fc2w_d = nc.dram_tensor("fc2wT", [8, 128, 4, C], F32R, kind="ExternalInput")
    id_d = nc.dram_tensor("ident", [128, 128], F32, kind="ExternalInput")
    if ln2_apply:
        lnw_d = nc.dram_tensor("ln2w", [1, C], F32, kind="ExternalInput")
        lnb_d = nc.dram_tensor("ln2b", [1, C], F32, kind="ExternalInput")
    out_d = nc.dram_tensor("out", [TS, C], F32, kind="ExternalOutput")

    n_ttiles = TS // 128    # 4

    with tile.TileContext(nc) as tc, ExitStack() as ctx:
        consts = ctx.enter_context(tc.tile_pool(name="consts", bufs=1))
        ident = consts.tile([128, 128], F32)
        nc.sync.dma_start(out=ident[:], in_=id_d[:])
        eps_sb = consts.tile([128, 1], F32)
        nc.vector.memset(eps_sb[:], 1e-5)
        pb_row = consts.tile([1, C], F32)
        nc.sync.dma_start(out=pb_row[:], in_=pb_d[:])
        pb_b = consts.tile([128, C], F32)
        nc.gpsimd.partition_broadcast(pb_b[:], pb_row[:])
        fb2_row = consts.tile([1, C], F32)
        nc.sync.dma_start(out=fb2_row[:], in_=fb2_d[:])
        fb2_b = consts.tile([128, C], F32)
        nc.gpsimd.partition_broadcast(fb2_b[:], fb2_row[:])
        ab_sb = consts.tile([128, 32], F32)
        nc.sync.dma_start(out=ab_sb[:], in_=ab_d[:])
        if ln2_apply:
            lnw_row = consts.tile([1, C], F32)
            nc.sync.dma_start(out=lnw_row[:], in_=lnw_d[:])
            lnw_b = consts.tile([128, C], F32)
            nc.gpsimd.partition_broadcast(lnw_b[:], lnw_row[:])
            lnb_row = consts.tile([1, C], F32)
            nc.sync.dma_start(out=lnb_row[:], in_=lnb_d[:])
            lnb_b = consts.tile([128, C], F32)
            nc.gpsimd.partition_broadcast(lnb_b[:], lnb_row[:])

        big = ctx.enter_context(tc.tile_pool(name="big", bufs=1))
        h2T = big.tile([128, 8, TS], F32R)         # 16KB/p
        x2pb = big.tile([128, n_ttiles, C], F32)   # x2 + fc2 bias, 16KB/p
        actT = big.tile([128, 32, TS], F32R)       # 64KB/p

        # ---- P1: reduce partials, LN2, transpose ----
        with tc.tile_pool(name="q1", bufs=3) as q1, \
             tc.tile_pool(name="q1s", bufs=4) as q1s, \
             tc.tile_pool(name="q1psum", bufs=4, space="PSUM") as q1p:
            for tt in range(n_ttiles):
                x2 = q1.tile([128, C], F32, tag="x2")
                g0 = q1.tile([128, C], F32, tag="g")
                nc.sync.dma_start(out=g0[:], in_=yp4_d[0, tt * 128:(tt + 1) * 128, :])
                g1 = q1.tile([128, C], F32, tag="g")
                nc.sync.dma_start(out=g1[:], in_=yp4_d[1, tt * 128:(tt + 1) * 128, :])
                nc.vector.tensor_add(x2[:], g0[:], g1[:])
                g2 = q1.tile([128, C], F32, tag="g")
                nc.sync.dma_start(out=g2[:], in_=yp4_d[2, tt * 128:(tt + 1) * 128, :])
                nc.vector.tensor_add(x2[:], x2[:], g2[:])
                g3 = q1.tile([128, C], F32, tag="g")
                nc.sync.dma_start(out=g3[:], in_=yp4_d[3, tt * 128:(tt + 1) * 128, :])
                nc.vector.tensor_add(x2[:], x2[:], g3[:])
                xst = q1.tile([128, C], F32, tag="xs")
                nc.sync.dma_start(out=xst[:], in_=xs_d[tt * 128:(tt + 1) * 128, :])
                nc.vector.tensor_add(x2[:], x2[:], xst[:])
                nc.vector.tensor_add(x2[:], x2[:], pb_b[:])
                nc.vector.tensor_add(x2pb[:, tt, :], x2[:], fb2_b[:])
                stats = q1s.tile([128, 2, 6], F32)
                x2g = x2[:].rearrange("p (g d) -> p g d", g=2)
                nc.vector.bn_stats(out=stats[:, 0, :], in_=x2g[:, 0, :])
                nc.vector.bn_stats(out=stats[:, 1, :], in_=x2g[:, 1, :])
                mv = q1s.tile([128, 2], F32)
                nc.vector.bn_aggr(out=mv[:], in_=stats[:])
                sd = q1s.tile([128, 1], F32, tag="sd")
                nc.scalar.activation(out=sd[:], in_=mv[:, 1:2], func=AF.Sqrt,
                                     bias=eps_sb[:], scale=1.0)
                rstd = q1s.tile([128, 1], F32)
                nc.vector.reciprocal(rstd[:], sd[:])
                h2 = q1.tile([128, C], F32, tag="h2")
                nc.vector.tensor_scalar(out=h2[:], in0=x2[:],
                                        scalar1=mv[:, 0:1], scalar2=rstd[:],
                                        op0=ALU.subtract, op1=ALU.mult)
                if ln2_apply:
                    nc.vector.tensor_mul(h2[:], h2[:], lnw_b[:])
                    nc.vector.tensor_add(h2[:], h2[:], lnb_b[:])
                for c in range(8):
                    pt = q1p.tile([128, 128], F32)
                    nc.tensor.transpose(pt[:], h2[:, c * 128:(c + 1) * 128], ident[:])
                    nc.vector.tensor_copy(h2T[:, c, tt * 128:(tt + 1) * 128], pt[:])

        # ---- P2: c_fc + gaussian activation (feature-major) ----
        with tc.tile_pool(name="q2w", bufs=2) as q2w, \
             tc.tile_pool(name="q2t", bufs=3) as q2t, \
             tc.tile_pool(name="q2psum", bufs=3, space="PSUM") as q2p:
            for hc in range(8):
                wt = q2w.tile([128, 8, 512], F32R)
                nc.sync.dma_start(out=wt[:], in_=fcw_d[hc])
                for ht in range(4):
                    pu = q2p.tile([128, TS], F32)
                    for c in range(8):
                        nc.tensor.matmul(
                            pu[:], wt[:, c, ht * 128:(ht + 1) * 128],
                            h2T[:, c, :], start=(c == 0), stop=(c == 7))
                    hi = hc * 4 + ht
                    usq = q2t.tile([128, TS], F32)
                    nc.scalar.activation(out=usq[:], in_=pu[:], func=AF.Square,
                                         bias=ab_sb[:, hi:hi + 1], scale=s_act)
                    nc.scalar.activation(out=actT[:, hi, :], in_=usq[:],
                                         func=AF.Exp, scale=-1.0)

        # ---- P3: c_fc2 + residual ----
        with tc.tile_pool(name="q3w", bufs=2) as q3w, \
             tc.tile_pool(name="q3o", bufs=3) as q3o, \
             tc.tile_pool(name="q3psum", bufs=1, space="PSUM") as q3p:
            po_tiles = []
            for tt in range(n_ttiles):
                row = []
                for co in range(2):
                    po_t = q3p.tile([128, 512], F32, tag=f"o{tt}{co}",
                                    name=f"po{tt}{co}")
                    row.append(po_t)
                po_tiles.append(row)
            for kr in range(8):
                w2 = q3w.tile([128, 4, C], F32R)
                nc.sync.dma_start(out=w2[:], in_=fc2w_d[kr])
                for tt in range(n_ttiles):
                    for k4 in range(4):
                        k = kr * 4 + k4
                        for co in range(2):
                            nc.tensor.matmul(
                                po_tiles[tt][co][:],
                                actT[:, k, tt * 128:(tt + 1) * 128],
                                w2[:, k4, co * 512:(co + 1) * 512],
                                start=(kr == 0 and k4 == 0),
                                stop=(kr == 7 and k4 == 3),
                                skip_group_check=True)
            for tt in range(n_ttiles):
                for co in range(2):
                    ot = q3o.tile([128, 512], F32)
                    nc.vector.tensor_add(ot[:], po_tiles[tt][co][:],
                                         x2pb[:, tt, co * 512:(co + 1) * 512])
                    nc.sync.dma_start(
                        out=out_d[tt * 128:(tt + 1) * 128,
                                  co * 512:(co + 1) * 512], in_=ot[:])
    nc.compile()
    return nc


# --------------------------------------------------------------------------
# Host-side orchestration
# --------------------------------------------------------------------------
_PROG_CACHE = {}


def _get_prog(key, builder, *args):
    if key not in _PROG_CACHE:
        _PROG_CACHE[key] = builder(*args)
    return _PROG_CACHE[key]


def _causal_masks4():
    s = np.arange(128)[:, None]
    t = np.arange(512)[None, :]
    ms = [((s + 128 * m) <= t).astype(np.float32) for m in range(4)]
    return np.ascontiguousarray(np.stack(ms, axis=1))  # [128, 4, 512]


def _perm(w, tiles, width):
    """[tiles*128, width] -> [128, tiles, width] (partition-major for DMA)."""
    return np.ascontiguousarray(w.reshape(tiles, 128, width).transpose(1, 0, 2))


def kernel(x, ln1_w, ln1_b, attn_w, attn_b, proj_w, proj_b,
           ln2_w, ln2_b, fc_w, fc_b, fc2_w, fc2_b,
           mu, sigma, gamma, beta, n_head):
    x = np.asarray(x, dtype=np.float32)
    attn_w = np.asarray(attn_w, dtype=np.float32)
    attn_b = np.asarray(attn_b, dtype=np.float32)
    proj_w = np.asarray(proj_w, dtype=np.float32)
    proj_b = np.asarray(proj_b, dtype=np.float32)
    fc_w = np.asarray(fc_w, dtype=np.float32)
    fc_b = np.asarray(fc_b, dtype=np.float32)
    fc2_w = np.asarray(fc2_w, dtype=np.float32)
    fc2_b = np.asarray(fc2_b, dtype=np.float32)
    ln1_w = np.asarray(ln1_w, dtype=np.float32)
    ln1_b = np.asarray(ln1_b, dtype=np.float32)
    ln2_w = np.asarray(ln2_w, dtype=np.float32)
    ln2_b = np.asarray(ln2_b, dtype=np.float32)
    mu = float(mu)
    sigma = float(sigma)
    gamma = float(gamma)
    beta = float(beta)
    n_head = int(n_head)

    B = x.shape[0]
    assert x.shape == (B, T, C) and B == 2 and n_head == 16

    _install_compile_cache()
    trace = bool(int(os.environ.get("BASS_KERNEL_TRACE", "0")))

    ln1_apply = not (np.allclose(ln1_w, 1.0) and np.allclose(ln1_b, 0.0))
    ln2_apply = not (np.allclose(ln2_w, 1.0) and np.allclose(ln2_b, 0.0))
    sig = abs(sigma) + 1e-8
    s_act = float(1.0 / (np.sqrt(2.0) * sig))

    # ---- launch 1 ----
    nc1 = _get_prog(("l1", ln1_apply), build_l1, ln1_apply)
    masks = _causal_masks4()
    vones = np.ones((128, 64), dtype=np.float32)
    in_maps1 = []
    for c in range(N_CORES):
        b, hg = c // 4, c % 4
        q_rows = attn_w[hg * 256:(hg + 1) * 256]
        k_rows = attn_w[C + hg * 256:C + (hg + 1) * 256]
        v_rows = attn_w[2 * C + hg * 256:2 * C + (hg + 1) * 256]
        wqkT = np.concatenate([q_rows, k_rows], axis=0).T  # [1024, 512]
        bqk = np.concatenate([attn_b[hg * 256:(hg + 1) * 256],
                              attn_b[C + hg * 256:C + (hg + 1) * 256]])
        m = {
            "xT": _perm(np.ascontiguousarray(x[b].T), 8, T),
            "wqkT": _perm(wqkT, 8, 512),
            "wvT": _perm(np.ascontiguousarray(v_rows.T), 8, 256),
            "bqk": np.ascontiguousarray(bqk.reshape(4, 128).T),
            "bv": np.ascontiguousarray(
                attn_b[2 * C + hg * 256:2 * C + (hg + 1) * 256][None, :]),
            "projwT": _perm(
                np.ascontiguousarray(proj_w[:, hg * 256:(hg + 1) * 256].T), 2, 1024),
            "masks": masks,
            "vones": vones,
        }
        if ln1_apply:
            m["ln1w"] = np.ascontiguousarray(ln1_w.reshape(8, 128).T)
            m["ln1b"] = np.ascontiguousarray(ln1_b.reshape(8, 128).T)
        in_maps1.append(m)
    res1 = run_bass_kernel_spmd(nc1, in_maps1, list(range(N_CORES)), trace=trace)
    if res1.exec_time_ns is not None:
        LAST_EXEC_NS["l1"] = res1.exec_time_ns
    yps = [res1.results[c]["yp"] for c in range(N_CORES)]

    # ---- launch 2 ----
    nc2 = _get_prog(("l2", ln2_apply, s_act), build_l2, ln2_apply, s_act)
    fc2w_eff = (gamma * fc2_w).T                        # [4096, 1024]
    fb2_eff = fc2_b + beta * fc2_w.sum(axis=1)
    abias = ((fc_b - mu) * s_act).reshape(32, 128).T    # [128, 32]
    fcwT_p = _perm(fc_w.T, 8, HID)                      # [128, 8, 4096]
    fcw_chunks = np.ascontiguousarray(
        fcwT_p.reshape(128, 8, 8, 512).transpose(2, 0, 1, 3))   # [8,128,8,512]
    fc2wT_p = _perm(np.ascontiguousarray(fc2w_eff), 32, C)      # [128, 32, 1024]
    fc2w_chunks = np.ascontiguousarray(
        fc2wT_p.reshape(128, 8, 4, C).transpose(1, 0, 2, 3))    # [8,128,4,1024]
    ident = np.eye(128, dtype=np.float32)
    in_maps2 = []
    for c in range(N_CORES):
        b, sl = c // 4, c % 4
        t0 = sl * TS
        yp4 = np.stack([yps[b * 4 + g][t0:t0 + TS] for g in range(4)])
        m = {
            "yp4": np.ascontiguousarray(yp4),
            "xs": np.ascontiguousarray(x[b, t0:t0 + TS]),
            "pb": proj_b[None, :],
            "fb2": np.ascontiguousarray(fb2_eff[None, :]),
            "abias": np.ascontiguousarray(abias),
            "fcwT": fcw_chunks,
            "fc2wT": fc2w_chunks,
            "ident": ident,
        }
        if ln2_apply:
            m["ln2w"] = ln2_w[None, :]
            m["ln2b"] = ln2_b[None, :]
        in_maps2.append(m)
    res2 = run_bass_kernel_spmd(nc2, in_maps2, list(range(N_CORES)), trace=trace)
    if res2.exec_time_ns is not None:
        LAST_EXEC_NS["l2"] = res2.exec_time_ns

    out = np.empty((B, T, C), dtype=np.float32)
    for c in range(N_CORES):
        b, sl = c // 4, c % 4
        out[b, sl * TS:(sl + 1) * TS] = res2.results[c]["out"]
    return out
